# revision 22
# baseline (speedup 1.0000x reference)
"""Trainium2 Bass kernel for nn_BasicBlock (dense_cnn, active-shift block).

Data-parallel over batch: 32 images -> 4 per NeuronCore across 8 cores.
Per-core layout: channels on SBUF partitions, pixels (H*W) on the free dim.

v2 restructure (from trace analysis of the v1 kernel, 82.5us):
  - x/prev are loaded CONTIGUOUS (rawx/rawp, casting f32->bf16 DMAs, split
    in image halves so conv1 of image 0 starts ~4us in instead of ~20us).
    bn1 becomes 4 partition-aligned tensor_scalar ops building the two
    conv1 group inputs; the group-1 lhsT rows are permuted to match.
    The residual matmul reads rawx directly -- the v1 xres SBUF->SBUF
    copies (~11us of DMA-pool time) are gone.
  - bn1+relu:  relu(s1*z + t1) = s1 * relu(z + t1/s1); s1 folded into w1.
  - conv1 (groups=2, bf16): two matmuls per pixel tile into a 112-partition
    padded PSUM layout (outputs must start at partition 0 or 64).
  - bn2+relu on VectorE (add+max tensor_scalar, 4x mode).
  - active_shift is separable bilinear, folds SWAPPED vs v1:
      * COLUMN pass on VectorE with the center tap normalized to 1
        (wc0/wc1, wc2/wc1 as tensor_scalar multipliers; wc1 folded into
        conv2 weights): 2 tensor_scalar (4x) + 2 tensor_tensor (2x) over
        per-row 3D APs + one 56-elem edge op.  The division is safe in
        bf16 (scale-free precision); wc1 is clamped to +-1e-4.
      * ROW pass folded into conv2: 3 matmuls with +-W-shifted flat rhs
        slices (no 3D APs, no wraparound fixups; first/last image row
        handled by restricting the AP range).
  - conv2 (groups=3) block-diagonal + residual accumulated in PSUM via an
    identity matmul from rawx; ScalarE evicts PSUM in 2-bank chunks.

dtype strategy: f32->bf16 casting load DMAs; bf16 compute; bf16 output DMAs
widened to f32 on the host.  End-to-end absmax-relative error ~6e-3.
"""

import os
import numpy as np
import ml_dtypes

import concourse.bass as bass
import concourse.bacc as bacc
import concourse.mybir as mybir
from concourse import tile
from concourse.bass_utils import run_bass_kernel_spmd

EPS = 1e-5
N_CORES = 8
N_PER = 4            # images per core
C = 96
CP = 112             # padded channel count for the post-conv1 layout
H = 56
W = 56
PIX = H * W          # 3136
RT = 7               # rows per spatial tile
TW = RT * W          # 392 pixels per tile (one PSUM bank each)
NT = H // RT         # 8 tiles per image
NPAIR = NT // 2      # 4 two-bank chunks per image
BANK = 512           # fp32 elems per PSUM bank
HALF = PIX // 2      # 1568

f32 = mybir.dt.float32
bf16 = mybir.dt.bfloat16

LAST_EXEC_NS = None


def _build_nc():
    nc = bacc.Bacc("TRN2", target_bir_lowering=False, debug=False, num_swdge_queues=4)

    x_ext = nc.declare_dram_parameter("x", [N_PER, C, PIX], f32, isOutput=False)
    p_ext = nc.declare_dram_parameter("prev", [N_PER, C, PIX], f32, isOutput=False)
    bias1_ext = nc.declare_dram_parameter("bias1", [CP, 2], f32, isOutput=False)
    t2_ext = nc.declare_dram_parameter("t2", [CP, 1], f32, isOutput=False)
    cc_ext = nc.declare_dram_parameter("cc", [CP, 2], f32, isOutput=False)
    w1t_ext = nc.declare_dram_parameter("w1t", [CP, CP], bf16, isOutput=False)
    w2x_ext = nc.declare_dram_parameter("w2x", [CP, 288], bf16, isOutput=False)
    resw_ext = nc.declare_dram_parameter("resw", [CP, C], bf16, isOutput=False)
    out_ext = nc.declare_dram_parameter("out", [N_PER, C, PIX], bf16, isOutput=True)
    fmap_ext = nc.declare_dram_parameter("fmap", [N_PER, C, PIX], bf16, isOutput=True)

    ADD = mybir.AluOpType.add
    MAX = mybir.AluOpType.max
    MULT = mybir.AluOpType.mult
    COPY = mybir.ActivationFunctionType.Copy

    with tile.TileContext(nc) as tc:
        with (
            tc.tile_pool(name="consts", bufs=1) as cpool,
            tc.tile_pool(name="raw", bufs=3) as rawp_pool,
            tc.tile_pool(name="act", bufs=2) as actp,
            tc.tile_pool(name="mid", bufs=3) as midp,
            tc.tile_pool(name="outs", bufs=2) as outp,
            tc.tile_pool(name="fpsum", bufs=2, space="PSUM") as fpsum,
            tc.tile_pool(name="opsum", bufs=2, space="PSUM") as opsum,
        ):
            bias1_sb = cpool.tile([CP, 2], f32)
            nc.sync.dma_start(out=bias1_sb[:], in_=bias1_ext[:])
            w1_sb = cpool.tile([CP, CP], bf16)
            nc.sync.dma_start(out=w1_sb[:], in_=w1t_ext[:])
            t2_sb = cpool.tile([CP, 1], f32)
            nc.sync.dma_start(out=t2_sb[:], in_=t2_ext[:])
            cc_sb = cpool.tile([CP, 2], f32)
            nc.sync.dma_start(out=cc_sb[:], in_=cc_ext[:])
            w2_sb = cpool.tile([CP, 288], bf16)
            nc.sync.dma_start(out=w2_sb[:], in_=w2x_ext[:])
            resw_sb = cpool.tile([CP, C], bf16)
            nc.sync.dma_start(out=resw_sb[:], in_=resw_ext[:])

            def emit_loads(n, halves):
                # Padded 112-partition layout (engine partition offsets must
                # be 0/64-aligned for >32-partition ranges): channels 0:48 at
                # partitions 0:48, channels 48:96 at partitions 64:112; the
                # [48:64] stripe carries duplicate channels 32:48 so every
                # partition is defined (zero-weighted in all matmuls).
                rawx = rawp_pool.tile([CP, PIX], bf16, tag="rawx", name=f"rawx{n}")
                rawp = rawp_pool.tile([CP, PIX], bf16, tag="rawp", name=f"rawp{n}")
                ranges = ((0, 2 * TW), (2 * TW, PIX)) if halves else ((0, PIX),)
                for h0, h1 in ranges:
                    hs = slice(h0, h1)
                    nc.gpsimd.dma_start(out=rawx[0:48, hs], in_=x_ext[n, 0:48, hs])
                    nc.gpsimd.dma_start(out=rawx[48:112, hs], in_=x_ext[n, 32:96, hs])
                    nc.gpsimd.dma_start(out=rawp[48:112, hs], in_=p_ext[n, 32:96, hs])
                    nc.gpsimd.dma_start(out=rawp[0:48, hs], in_=p_ext[n, 0:48, hs])
                return rawx, rawp

            def emit_bn1(n, rawx, rawp, halves):
                g0a = actp.tile([CP, PIX], bf16, tag="g0a", name=f"g0a{n}")
                g1a = actp.tile([CP, PIX], bf16, tag="g1a", name=f"g1a{n}")
                ranges = (((0, 2 * TW), (2 * TW, 4 * TW), (4 * TW, PIX))
                          if halves else ((0, PIX),))
                for h0, h1 in ranges:
                    hs = slice(h0, h1)
                    nc.vector.tensor_scalar(
                        g0a[0:64, hs], rawx[0:64, hs], bias1_sb[0:64, 0:1], 0.0,
                        ADD, MAX)
                    nc.vector.tensor_scalar(
                        g0a[64:112, hs], rawp[64:112, hs], bias1_sb[64:112, 0:1],
                        0.0, ADD, MAX)
                    nc.vector.tensor_scalar(
                        g1a[0:64, hs], rawp[0:64, hs], bias1_sb[0:64, 1:2], 0.0,
                        ADD, MAX)
                    nc.vector.tensor_scalar(
                        g1a[64:112, hs], rawx[64:112, hs], bias1_sb[64:112, 1:2],
                        0.0, ADD, MAX)
                return g0a, g1a

            def emit_conv1(n, g0a, g1a):
                # conv1 (groups=2) + fmap eviction + fmap stores
                fmap_sb = midp.tile([CP, PIX], bf16, tag="fmap",
                                    name=f"fmap{n}")
                for cth in range(NPAIR):
                    fp = fpsum.tile([CP, 2 * BANK], f32, tag="fp")
                    for k in range(2):
                        t = 2 * cth + k
                        sl = slice(t * TW, (t + 1) * TW)
                        pb = slice(k * BANK, k * BANK + TW)
                        nc.tensor.matmul(
                            fp[0:64, pb], w1_sb[:, 0:64],
                            g0a[:, sl], start=True, stop=True,
                        )
                        nc.tensor.matmul(
                            fp[64:112, pb], w1_sb[:, 64:112],
                            g1a[:, sl], start=True, stop=True,
                        )
                    fpv = fp.rearrange("p (b w) -> p b w", w=BANK)[:, :, 0:TW]
                    csl = slice(cth * 2 * TW, (cth + 1) * 2 * TW)
                    fv = fmap_sb[:, csl].rearrange("p (b w) -> p b w", w=TW)
                    nc.scalar.activation(fv, fpv, COPY)
                    if cth % 2 == 1:
                        hsl = slice((cth - 1) * 2 * TW, (cth + 1) * 2 * TW)
                        nc.sync.dma_start(out=fmap_ext[n, 0:48, hsl],
                                          in_=fmap_sb[0:48, hsl])
                        nc.sync.dma_start(out=fmap_ext[n, 48:96, hsl],
                                          in_=fmap_sb[64:112, hsl])
                return fmap_sb

            def emit_fold(n, fmap_sb, halves):
                # bn2 then column pass of the shift, center tap normalized:
                # v[x] = c0*b[x-1] + b[x] + c2*b[x+1]  (zero-padded row edges)
                b_sb = midp.tile([CP, PIX], bf16, tag="b", name=f"b{n}")
                t0_sb = midp.tile([CP, PIX], bf16, tag="t0", name=f"t0{n}")
                t2t_sb = midp.tile([CP, PIX], bf16, tag="t2t", name=f"t2t{n}")
                v_sb = midp.tile([CP, PIX], bf16, tag="v", name=f"v{n}")
                b3 = b_sb.rearrange("p (r w) -> p r w", w=W)
                t03 = t0_sb.rearrange("p (r w) -> p r w", w=W)
                t23 = t2t_sb.rearrange("p (r w) -> p r w", w=W)
                v3 = v_sb.rearrange("p (r w) -> p r w", w=W)
                ranges = ((0, H // 2), (H // 2, H)) if halves else ((0, H),)
                for r0, r1 in ranges:
                    hs = slice(r0 * W, r1 * W)
                    rs = slice(r0, r1)
                    nc.vector.tensor_scalar(
                        b_sb[:, hs], fmap_sb[:, hs], t2_sb[:, 0:1], 0.0,
                        ADD, MAX)
                    nc.vector.tensor_scalar(
                        t0_sb[:, hs], b_sb[:, hs], cc_sb[:, 0:1], None, MULT)
                    nc.vector.tensor_scalar(
                        t2t_sb[:, hs], b_sb[:, hs], cc_sb[:, 1:2], None, MULT)
                    nc.vector.tensor_tensor(
                        v3[:, rs, 1:W], b3[:, rs, 1:W], t03[:, rs, 0:W - 1],
                        ADD)
                    nc.vector.tensor_tensor(
                        v3[:, rs, 1:W - 1], v3[:, rs, 1:W - 1],
                        t23[:, rs, 2:W], ADD)
                    nc.vector.tensor_tensor(
                        v3[:, rs, 0:1], b3[:, rs, 0:1], t23[:, rs, 1:2], ADD)
                return v_sb

            def emit_conv2(n, v_sb, rawx, fine_stores=False):
                # conv2 (row taps as +-W shifted matmuls) + residual + stores
                out_sb = outp.tile([C, PIX], bf16, tag="out", name=f"out{n}")
                for cth in range(NPAIR):
                    op = opsum.tile([C, 2 * BANK], f32, tag="op")
                    for k in range(2):
                        t = 2 * cth + k
                        s0 = t * TW
                        p0 = k * BANK
                        nc.tensor.matmul(
                            op[:, p0:p0 + TW], w2_sb[:, 96:192],
                            v_sb[:, s0:s0 + TW],
                            start=True, stop=False, skip_group_check=True)
                        if t == 0:
                            nc.tensor.matmul(
                                op[:, p0 + W:p0 + TW], w2_sb[:, 0:96],
                                v_sb[:, 0:TW - W],
                                start=False, stop=False, skip_group_check=True)
                        else:
                            nc.tensor.matmul(
                                op[:, p0:p0 + TW], w2_sb[:, 0:96],
                                v_sb[:, s0 - W:s0 - W + TW],
                                start=False, stop=False, skip_group_check=True)
                        if t == NT - 1:
                            nc.tensor.matmul(
                                op[:, p0:p0 + TW - W], w2_sb[:, 192:288],
                                v_sb[:, s0 + W:PIX],
                                start=False, stop=False, skip_group_check=True)
                        else:
                            nc.tensor.matmul(
                                op[:, p0:p0 + TW], w2_sb[:, 192:288],
                                v_sb[:, s0 + W:s0 + W + TW],
                                start=False, stop=False, skip_group_check=True)
                        nc.tensor.matmul(
                            op[:, p0:p0 + TW], resw_sb[:], rawx[:, s0:s0 + TW],
                            start=False, stop=True, skip_group_check=True)
                    opv = op.rearrange("p (b w) -> p b w", w=BANK)[:, :, 0:TW]
                    csl = slice(cth * 2 * TW, (cth + 1) * 2 * TW)
                    ov = out_sb[:, csl].rearrange("p (b w) -> p b w", w=TW)
                    nc.scalar.activation(ov, opv, COPY)
                    if fine_stores:
                        nc.sync.dma_start(out=out_ext[n, :, csl],
                                          in_=out_sb[:, csl])
                    elif cth % 2 == 1:
                        hsl = slice((cth - 1) * 2 * TW, (cth + 1) * 2 * TW)
                        nc.sync.dma_start(out=out_ext[n, :, hsl],
                                          in_=out_sb[:, hsl])

            # Software pipeline.  Per-engine program order is the schedule
            # hint; each engine executes its stream IN ORDER, so conv1(n+1)
            # must precede conv2(n) in the PE stream (conv2(n) waits on
            # fold(n), which runs on VectorE while the PE does conv1(n+1)).
            st = {}
            st[0] = emit_loads(0, halves=True)
            st[0] += emit_bn1(0, st[0][0], st[0][1], halves=True)
            if N_PER > 1:
                st[1] = emit_loads(1, halves=False)
            fmaps = {0: emit_conv1(0, st[0][2], st[0][3])}
            if N_PER > 1:
                st[1] += emit_bn1(1, st[1][0], st[1][1], halves=False)
            for n in range(N_PER):
                if n + 2 < N_PER:
                    st[n + 2] = emit_loads(n + 2, halves=False)
                v_sb = emit_fold(n, fmaps[n], halves=(n in (0, N_PER - 1)))
                if n + 1 < N_PER:
                    fmaps[n + 1] = emit_conv1(n + 1, st[n + 1][2], st[n + 1][3])
                if n + 2 < N_PER:
                    st[n + 2] += emit_bn1(n + 2, st[n + 2][0], st[n + 2][1],
                                          halves=False)
                emit_conv2(n, v_sb, st[n][0], fine_stores=(n == N_PER - 1))

    nc.compile()
    return nc


def _prep_consts(bn1_gamma, bn1_beta, bn1_mean, bn1_var,
                 bn2_gamma, bn2_beta, bn2_mean, bn2_var, w1, w2, shift):
    s1 = bn1_gamma / np.sqrt(bn1_var + EPS)   # [192], per concat-fmap channel
    t1 = bn1_beta - bn1_mean * s1
    r1 = t1 / s1

    # Padded act layout: partitions 0:48 / 64:112 are real, 48:64 dead.
    # g0a = [x ch 0:48 | . | prev ch 48:96]  (concat ch 0..95)
    # g1a = [prev ch 0:48 | . | x ch 48:96]  (concat ch [144:192, 96:144])
    bias1 = np.zeros((CP, 2), np.float32)
    bias1[0:48, 0] = r1[0:48]
    bias1[64:112, 0] = r1[48:96]
    bias1[0:48, 1] = r1[144:192]
    bias1[64:112, 1] = r1[96:144]

    w1m = w1[:, :, 0, 0]  # (96 out, 96 in-per-group)
    w1t = np.zeros((CP, CP), np.float32)
    w1t[0:48, 0:48] = (w1m[0:48, 0:48] * s1[None, 0:48]).T
    w1t[64:112, 0:48] = (w1m[0:48, 48:96] * s1[None, 48:96]).T
    w1t[0:48, 64:112] = (w1m[48:96, 48:96] * s1[None, 144:192]).T
    w1t[64:112, 64:112] = (w1m[48:96, 0:48] * s1[None, 96:144]).T

    # padded partition index for conv1-output channel c
    pidx = np.concatenate([np.arange(48), 64 + np.arange(48)])  # [96]

    s2f = bn2_gamma / np.sqrt(bn2_var + EPS)
    b2f = bn2_beta - bn2_mean * s2f
    t2 = np.zeros((CP, 1), np.float32)
    t2[pidx, 0] = b2f / s2f

    dy, dx = shift[:, 0].astype(np.float64), shift[:, 1].astype(np.float64)
    ay, ax = np.floor(dy), np.floor(dx)
    fy, fx = dy - ay, dx - ax
    wrf = np.zeros((C, 3))
    wcf = np.zeros((C, 3))
    for c in range(C):
        iy = int(ay[c]) + 1   # tap index of row offset ay (offset = idx - 1)
        ix = int(ax[c]) + 1
        wrf[c, iy] += 1.0 - fy[c]
        wrf[c, iy + 1] += fy[c]
        wcf[c, ix] += 1.0 - fx[c]
        wcf[c, ix + 1] += fx[c]

    wc1 = wcf[:, 1]
    wc1c = np.where(np.abs(wc1) < 1e-4, np.copysign(1e-4, wc1), wc1)
    cc = np.zeros((CP, 2), np.float32)
    cc[pidx, 0] = wcf[:, 0] / wc1c
    cc[pidx, 1] = wcf[:, 2] / wc1c

    w2m = w2[:, :, 0, 0]  # (96 out, 32 in-per-group)
    w2full = np.zeros((C, C))
    for g in range(3):
        w2full[32 * g:32 * g + 32, 32 * g:32 * g + 32] = w2m[32 * g:32 * g + 32]
    # w2x[:, 96*oy + o]: row tap oy (offset oy-1), with s2*wc1 folded in
    w2x = np.zeros((CP, 288), np.float32)
    chscale = s2f * wc1c
    for oy in range(3):
        w2x[pidx, 96 * oy:96 * oy + 96] = (w2full.T * (wrf[:, oy] * chscale)[:, None])

    resw = np.zeros((CP, C), np.float32)
    resw[np.arange(48), np.arange(48)] = 1.0
    resw[64 + np.arange(48), 48 + np.arange(48)] = 1.0

    return {
        "bias1": bias1,
        "t2": t2,
        "cc": cc,
        "w1t": w1t.astype(ml_dtypes.bfloat16),
        "w2x": w2x.astype(ml_dtypes.bfloat16),
        "resw": resw.astype(ml_dtypes.bfloat16),
    }


_NC_CACHE = {}


def kernel(x, prev_fmap, bn1_gamma, bn1_beta, bn1_mean, bn1_var,
           bn2_gamma, bn2_beta, bn2_mean, bn2_var, w1, w2, shift):
    global LAST_EXEC_NS
    x = np.ascontiguousarray(np.asarray(x, np.float32))
    prev_fmap = np.ascontiguousarray(np.asarray(prev_fmap, np.float32))
    consts = _prep_consts(
        np.asarray(bn1_gamma, np.float32), np.asarray(bn1_beta, np.float32),
        np.asarray(bn1_mean, np.float32), np.asarray(bn1_var, np.float32),
        np.asarray(bn2_gamma, np.float32), np.asarray(bn2_beta, np.float32),
        np.asarray(bn2_mean, np.float32), np.asarray(bn2_var, np.float32),
        np.asarray(w1, np.float32), np.asarray(w2, np.float32),
        np.asarray(shift, np.float32))

    if "nc" not in _NC_CACHE:
        _NC_CACHE["nc"] = _build_nc()
    nc = _NC_CACHE["nc"]

    NB = x.shape[0]
    xs = x.reshape(N_CORES, N_PER, C, PIX)
    ps = prev_fmap.reshape(N_CORES, N_PER, C, PIX)
    in_maps = [
        {"x": xs[i], "prev": ps[i], **consts}
        for i in range(N_CORES)
    ]

    trace = bool(os.environ.get("CC_KERNEL_TRACE"))
    res = run_bass_kernel_spmd(
        nc, in_maps, core_ids=list(range(N_CORES)), trace=trace,
    )
    LAST_EXEC_NS = res.exec_time_ns

    out = np.empty((NB, C, PIX), np.float32)
    fmap = np.empty((NB, C, PIX), np.float32)
    for i in range(N_CORES):
        out[i * N_PER:(i + 1) * N_PER] = res.results[i]["out"].astype(np.float32)
        fmap[i * N_PER:(i + 1) * N_PER] = res.results[i]["fmap"].astype(np.float32)
    return (out.reshape(NB, C, H, W), fmap.reshape(NB, C, H, W))
